# revision 14
# baseline (speedup 1.0000x reference)
"""Trainium2 Bass kernel for CausalSelfAttention (full softmax + RoPE).

Problem: x[4,2048,2048] -> qkv proj (W_attn [6144,2048]) -> RoPE(q,k) ->
softmax(q k^T / sqrt(128)) v -> out proj (W_proj [2048,2048]).

Sharding: 8 cores = (batch 4) x (head-group 2). Core c=(b,hg) computes heads
hg*8..hg*8+7 of batch b and the partial output projection over those heads'
columns; host sums the two partials per batch.

v2 vs baseline: all matmul operands in bf16 (PE rate is 1 cycle/row either
way, but DMA+SBUF halve and LDWEIGHTS gets FWL); K^T and V stay resident in
SBUF (no DRAM scratch round-trip, V/K written by ACT/DVE directly); softmax
denominators inverted on ACT via exp(-ln(x)) instead of the slow DVE
reciprocal; the attention kt-loop is software-pipelined two steps so the
ACT exp latency never stalls the PE.

Per-core pipeline (all matmuls bf16 -> fp32 PSUM):
  stage 1a: V = x @ Wv^T in [t, d] layout -> SBUF resident
  stage 1b: K^T = RoPE(Wk x^T) -> SBUF resident; Q^T -> DRAM scratch (bf16)
  stage 2:  per (head, q-chunk): S^T tiles = K'^T(stationary) @ Q' -> exp
            (ACT, no max subtraction: |logits| <~ 6) -> PV (V stationary) and
            column sums (ones stationary) -> denom^-1 = exp(-ln(sum)) on ACT
            -> DVE mult -> pvt (bf16)
  stage 3:  OT[c',t] partial = Wp^T @ PVT over this core's 1024 hd columns
"""
import sys
for _p in ('/opt/pypackages', '/opt/trn_rl_repo'):
    if _p not in sys.path:
        sys.path.insert(0, _p)

from contextlib import ExitStack

import numpy as np
import ml_dtypes

import concourse.bacc as bacc
import concourse.tile as tile
from concourse import mybir
from concourse.bass_utils import run_bass_kernel_spmd

# Both Exp and Ln are used in the inner attention loop. Left alone, the
# table-load pass binds Exp to `exp_and_others` and Ln to `natural_log`,
# inserting a ~2.7us table switch per q-chunk (~170us/kernel). Restrict
# Exp/Ln to the combined set so one load at kernel start covers both.
_orig_get_tables = bacc.get_activation_tables


def _combined_exp_ln_tables(arch):
    both = {mybir.ActivationFunctionType.Exp, mybir.ActivationFunctionType.Ln}
    out = {}
    for name, funcs in _orig_get_tables(arch).items():
        if name != "natural_log_exp_and_others" and (both & funcs):
            funcs = funcs - both
        out[name] = funcs
    return out


bacc.get_activation_tables = _combined_exp_ln_tables

F32 = mybir.dt.float32
BF16 = mybir.dt.bfloat16
BF16_NP = ml_dtypes.bfloat16

B, T, C = 4, 2048, 2048
H, D = 16, 128
HPC = 8                 # heads per core
FV = HPC * D            # 1024
SCALE = 1.0 / float(np.sqrt(D))
N_CORES = 8


def build_nc(t=T, reps=1, stages=(1, 2, 3)):
    assert t % 512 == 0
    n_tt = t // 128      # t-tiles of 128
    n_tc = t // 512      # t-chunks of 512
    KC = C // 128        # contraction chunks over C

    nc = bacc.Bacc("TRN2", target_bir_lowering=False)

    xT_d = nc.dram_tensor("xT", [C, t], BF16, kind="ExternalInput")
    wqk_d = nc.dram_tensor("wqk4", [16, KC, 128, 128], BF16, kind="ExternalInput")
    wv_d = nc.dram_tensor("wv3", [KC, 128, FV], BF16, kind="ExternalInput")
    wp_d = nc.dram_tensor("wp4", [HPC, 16, 128, 128], BF16, kind="ExternalInput")
    cos_d = nc.dram_tensor("cosT", [128, t], F32, kind="ExternalInput")
    sin_d = nc.dram_tensor("sinS", [128, t], F32, kind="ExternalInput")
    ones_d = nc.dram_tensor("ones", [128, 128], BF16, kind="ExternalInput")
    ot_d = nc.dram_tensor("OT", [C, t], F32, kind="ExternalOutput")

    with tile.TileContext(nc) as tc, ExitStack() as octx:
        if reps > 1:
            octx.enter_context(tc.For_i(0, reps, 1))
        const = octx.enter_context(tc.tile_pool(name="const", bufs=1))
        ones_t = const.tile([128, 128], BF16, name="ones_t")
        nc.sync.dma_start(out=ones_t, in_=ones_d.ap())

        # warm the ACT table (ln+exp combined set) before stage 2 needs it
        warm = const.tile([128, 8], F32, name="warm")
        nc.vector.memset(warm, 1.0)
        warm2 = const.tile([128, 8], F32, name="warm2")
        nc.scalar.activation(warm2, warm, mybir.ActivationFunctionType.Exp)
        nc.scalar.activation(warm2, warm, mybir.ActivationFunctionType.Ln)

        # K^T and V live in SBUF for the whole kernel
        kvp = octx.enter_context(tc.tile_pool(name="kvp", bufs=1))
        k_sb = [kvp.tile([128, t], BF16, name=f"k_sb{h}") for h in range(HPC)]
        v_sb = kvp.tile([128, n_tt, FV], BF16, name="v_sb")

        dpool = octx.enter_context(tc.tile_pool(name="scratch", bufs=1, space="DRAM"))
        q_scr = [dpool.tile([128, t], BF16, name=f"q_scr{h}") for h in range(HPC)]

        # ------- stage 1: xT resident; V from xT-slices (stationary), then QK -------
        with ExitStack() as ctx:
            resid = ctx.enter_context(tc.tile_pool(name="resid", bufs=1))
            rope = ctx.enter_context(tc.tile_pool(name="rope", bufs=3))
            stg = ctx.enter_context(tc.tile_pool(name="stg1", bufs=4))
            ps1 = ctx.enter_context(tc.tile_pool(name="ps1", bufs=8, space="PSUM"))

            # V weight half 0 loads FIRST so PE can start without waiting for
            # the full xT upload to clear the DMA queues.
            with ExitStack() as vctx:
                wvp = vctx.enter_context(tc.tile_pool(name="wvp", bufs=1))
                wv_t = []
                for kc in range(KC):
                    w = wvp.tile([128, 512], BF16, name=f"wv0_{kc}",
                                 tag=f"wv{kc}")
                    nc.sync.dma_start(out=w, in_=wv_d.ap()[kc][:, 0:512])
                    wv_t.append(w)

                # xT arrives t-chunk-major so the first V/QK tiles unblock
                # after ~1/4 of the upload instead of all of it
                xt_t = [resid.tile([128, t], BF16, name=f"xt{kc}")
                        for kc in range(KC)]
                n_xch = max(1, t // 512)
                for th in range(n_xch):
                    sl = slice(th * (t // n_xch), (th + 1) * (t // n_xch))
                    for kc in range(KC):
                        nc.sync.dma_start(
                            out=xt_t[kc][:, sl],
                            in_=xT_d.ap()[kc * 128:(kc + 1) * 128, sl])
                cos_t = resid.tile([128, t], F32, name="cos_t")
                nc.sync.dma_start(out=cos_t, in_=cos_d.ap())
                sin_t = resid.tile([128, t], F32, name="sin_t")
                nc.sync.dma_start(out=sin_t, in_=sin_d.ap())

                # V: psum [t-tile, f-half] = sum_kc xT[kc, t-tile].T @ WvT
                for fh in range(FV // 512):
                    if fh > 0:
                        wv_t = []
                        for kc in range(KC):
                            w = wvp.tile([128, 512], BF16, name=f"wv{fh}_{kc}",
                                         tag=f"wv{kc}")
                            nc.sync.dma_start(
                                out=w,
                                in_=wv_d.ap()[kc][:, fh * 512:(fh + 1) * 512])
                            wv_t.append(w)
                    for tt in range(n_tt):
                        ps = ps1.tile([128, 512], F32, tag="ps")
                        for kc in range(KC):
                            nc.tensor.matmul(
                                ps,
                                lhsT=xt_t[kc][:, tt * 128:(tt + 1) * 128],
                                rhs=wv_t[kc],
                                start=(kc == 0), stop=(kc == KC - 1))
                        nc.scalar.copy(
                            v_sb[:, tt, fh * 512:(fh + 1) * 512], ps)

            # QK in head-paired order (k first) so stage 2 head h unblocks
            # early
            wqkp = ctx.enter_context(tc.tile_pool(name="wqkp", bufs=2))
            for ft in [x for h in range(HPC) for x in (h + HPC, h)]:
                wq = wqkp.tile([128, KC, 128], BF16, tag="wq")
                nc.sync.dma_start(
                    out=wq, in_=wqk_d.ap()[ft].rearrange("kc p f -> p kc f"))
                is_k = ft >= HPC
                h = ft - HPC if is_k else ft
                for tch in range(n_tc):
                    sl = slice(tch * 512, (tch + 1) * 512)
                    ps = ps1.tile([128, 512], F32, tag="ps")
                    for kc in range(KC):
                        nc.tensor.matmul(ps, lhsT=wq[:, kc, :],
                                         rhs=xt_t[kc][:, sl],
                                         start=(kc == 0), stop=(kc == KC - 1))
                    # RoPE: out = ps*cos + rot(ps)*sin  (sin pre-shifted+signed)
                    tmp = rope.tile([128, 512], F32, tag="tmp")
                    nc.vector.tensor_tensor(out=tmp[0:64, :], in0=ps[64:128, :],
                                            in1=sin_t[64:128, sl],
                                            op=mybir.AluOpType.mult)
                    nc.vector.tensor_tensor(out=tmp[64:128, :], in0=ps[0:64, :],
                                            in1=sin_t[0:64, sl],
                                            op=mybir.AluOpType.mult)
                    qc_t = rope.tile([128, 512], F32, tag="qc")
                    nc.vector.tensor_tensor(out=qc_t, in0=ps, in1=cos_t[:, sl],
                                            op=mybir.AluOpType.mult)
                    if is_k:
                        nc.vector.tensor_tensor(out=k_sb[h][:, sl], in0=qc_t,
                                                in1=tmp,
                                                op=mybir.AluOpType.add)
                    else:
                        st = stg.tile([128, 512], BF16, tag="st")
                        nc.vector.tensor_tensor(out=st, in0=qc_t, in1=tmp,
                                                op=mybir.AluOpType.add)
                        nc.sync.dma_start(out=q_scr[h][:, sl], in_=st)

        # ---------------- stages 2+3 share the persistent PVT ----------------
        with ExitStack() as octx2:
          if 2 in stages:
            pvtp = octx2.enter_context(tc.tile_pool(name="pvtp", bufs=1))
            pvt = [pvtp.tile([128, t], BF16, name=f"pvt{h}") for h in range(HPC)]
            wpp = octx2.enter_context(tc.tile_pool(name="wpp", bufs=1))
            wp_t = [wpp.tile([128, 16, 128], BF16, name=f"wp{hc}")
                    for hc in range(HPC)]

            # ---------------- stage 2: attention per head ----------------
            # All 4 q-chunks advance together through the kt loop so the four
            # softmax-sum matmuls pack into ONE concurrent col-tiled group
            # ([128,32] ones at col groups 0..3 -> 4 rhs streams in ~1 matmul
            # span; measured 352ns vs 4x258ns separate). The shared sum bank
            # holds denom(qc) at partitions [32qc,32qc+32); a 1/32-ones
            # broadcast matmul re-expands each to all 128 partitions for the
            # final DVE mult.
            with ExitStack() as ctx:
                qkv_io = ctx.enter_context(tc.tile_pool(name="qkv_io", bufs=2))
                expp = ctx.enter_context(tc.tile_pool(name="expp", bufs=9))
                recp = ctx.enter_context(tc.tile_pool(name="recp", bufs=2))
                pss = ctx.enter_context(tc.tile_pool(name="pss", bufs=3, space="PSUM"))
                pspv = ctx.enter_context(tc.tile_pool(name="pspv", bufs=1, space="PSUM"))
                pssm = ctx.enter_context(tc.tile_pool(name="pssm", bufs=1, space="PSUM"))

                inv_t = const.tile([128, 128], BF16, name="inv_t")
                nc.vector.memset(inv_t, 1.0 / 32.0)

                for h in range(HPC):
                    qh = qkv_io.tile([128, t], BF16, tag="qh")
                    nc.scalar.dma_start(out=qh, in_=q_scr[h])
                    kh = k_sb[h]
                    vh = v_sb[:, :, h * 128:(h + 1) * 128]
                    if h == 1:
                        # stream Wp in while attention compute hides it
                        for hc in range(HPC):
                            nc.scalar.dma_start(
                                out=wp_t[hc],
                                in_=wp_d.ap()[hc].rearrange("ct p f -> p ct f"))

                    ps_pv = [pspv.tile([128, 512], F32, tag=f"pv{qc}", name=f"pv{qc}")
                             for qc in range(n_tc)]
                    ps_sm = pssm.tile([128, 512], F32, tag="smbc")

                    def emit_pv(pkt, pe, qc):
                        nc.tensor.matmul(ps_pv[qc], lhsT=vh[:, pkt, :],
                                         rhs=pe[qc],
                                         start=(pkt == 0),
                                         stop=(pkt == n_tt - 1))

                    def emit_sums(pkt, pe):
                        for qc in range(n_tc):
                            nc.tensor.matmul(
                                ps_sm[32 * qc:32 * (qc + 1), :],
                                lhsT=ones_t[:, 32 * qc:32 * (qc + 1)],
                                rhs=pe[qc],
                                start=(pkt == 0 and qc == 0),
                                stop=(pkt == n_tt - 1),
                                tile_position=(0, 32 * qc))

                    def emit_s(kt, qc):
                        sl = slice(qc * 512, (qc + 1) * 512)
                        ps_s = pss.tile([128, 512], F32, tag="pss")
                        nc.tensor.matmul(ps_s,
                                         lhsT=kh[:, kt * 128:(kt + 1) * 128],
                                         rhs=qh[:, sl],
                                         start=True, stop=True)
                        e = expp.tile([128, 512], BF16, tag="e")
                        nc.scalar.activation(e, ps_s,
                                             mybir.ActivationFunctionType.Exp,
                                             scale=SCALE)
                        return e

                    # per kt: S matmuls for the 4 q-chunks, interleaved with
                    # the previous kt's PV + packed-sum matmuls so neither the
                    # exp latency nor the 3-bank S rotation ever stalls PE
                    prev = None
                    for kt in range(n_tt):
                        es = []
                        pv_q = list(range(n_tc)) if prev is not None else []
                        for qc in range(n_tc):
                            es.append(emit_s(kt, qc))
                            if qc >= 1:
                                for _ in range(2):
                                    if pv_q:
                                        emit_pv(kt - 1, prev, pv_q.pop(0))
                        if prev is not None:
                            while pv_q:
                                emit_pv(kt - 1, prev, pv_q.pop(0))
                            emit_sums(kt - 1, prev)
                        prev = es
                    for qc in range(n_tc):
                        emit_pv(n_tt - 1, prev, qc)
                    emit_sums(n_tt - 1, prev)

                    # denom^-1 = exp(-ln(denom)) on ACT, then per-qc broadcast
                    # to 128 partitions via a [32,128] 1/32-ones matmul
                    lnt = recp.tile([128, 512], F32, tag="lnt")
                    nc.scalar.activation(lnt, ps_sm,
                                         mybir.ActivationFunctionType.Ln)
                    rec = recp.tile([128, 512], BF16, tag="rec")
                    nc.scalar.activation(rec, lnt,
                                         mybir.ActivationFunctionType.Exp,
                                         scale=-1.0)
                    for qc in range(n_tc):
                        sl = slice(qc * 512, (qc + 1) * 512)
                        ps_bc = pssm.tile([128, 512], F32, tag="smbc")
                        nc.tensor.matmul(
                            ps_bc,
                            lhsT=inv_t[32 * qc:32 * (qc + 1), :],
                            rhs=rec[32 * qc:32 * (qc + 1), :],
                            start=True, stop=True,
                            tile_position=(32 * qc, 0))
                        bc_sb = recp.tile([128, 512], F32, tag="bc_sb")
                        nc.scalar.copy(bc_sb, ps_bc)
                        nc.vector.tensor_tensor(out=pvt[h][:, sl],
                                                in0=ps_pv[qc], in1=bc_sb,
                                                op=mybir.AluOpType.mult)

            # ---------------- stage 3: output projection ----------------
            with ExitStack() as ctx:
              if 3 in stages:
                ostg = ctx.enter_context(tc.tile_pool(name="ostg", bufs=4))
                ps3 = ctx.enter_context(tc.tile_pool(name="ps3", bufs=4, space="PSUM"))

                for tch in range(n_tc):
                    sl = slice(tch * 512, (tch + 1) * 512)
                    for ct in range(16):
                        ps = ps3.tile([128, 512], F32, tag="ps")
                        for hc in range(HPC):
                            nc.tensor.matmul(ps, lhsT=wp_t[hc][:, ct, :],
                                             rhs=pvt[hc][:, sl],
                                             start=(hc == 0), stop=(hc == HPC - 1))
                        st = ostg.tile([128, 512], F32, tag="st")
                        nc.scalar.copy(st, ps)
                        nc.sync.dma_start(
                            out=ot_d.ap()[ct * 128:(ct + 1) * 128, sl], in_=st)

        if stages != (1, 2, 3):
            # timing-probe build: keep OT written so outputs exist
            dummy = const.tile([128, 128], F32, name="dummy_ot")
            nc.vector.memset(dummy, 0.0)
            nc.sync.dma_start(out=ot_d.ap()[0:128, 0:128], in_=dummy)

    nc.compile()
    return nc


def make_in_maps(x, cos, sin, W_attn, W_proj):
    t = x.shape[1]
    KC = C // 128
    x = np.asarray(x, np.float32)
    cosT = np.ascontiguousarray(np.asarray(cos, np.float32)[0].T)        # [D, t]
    sinT = np.asarray(sin, np.float32)[0].T                               # [D, t]
    sinS = np.ascontiguousarray(
        np.concatenate([sinT[64:128], -sinT[0:64]], axis=0))
    ones = np.ones((128, 128), BF16_NP)
    W_attn = np.asarray(W_attn, np.float32)
    W_proj = np.asarray(W_proj, np.float32)

    xT_b = [np.ascontiguousarray(x[b].T.astype(BF16_NP)) for b in range(B)]

    per_hg = []
    for hg in range(2):
        r = slice(hg * 1024, (hg + 1) * 1024)
        wq = W_attn[0 * C + hg * 1024:0 * C + (hg + 1) * 1024]
        wk = W_attn[1 * C + hg * 1024:1 * C + (hg + 1) * 1024]
        wv = W_attn[2 * C + hg * 1024:2 * C + (hg + 1) * 1024]
        wqkT = np.concatenate([wq, wk], axis=0).T                         # [C, 2048]
        wqk4 = np.ascontiguousarray(
            wqkT.reshape(KC, 128, 16, 128).transpose(2, 0, 1, 3).astype(BF16_NP))
        wv3 = np.ascontiguousarray(wv.T.reshape(KC, 128, FV).astype(BF16_NP))
        wpT = W_proj[:, r].T                                              # [1024, C]
        wp4 = np.ascontiguousarray(
            wpT.reshape(HPC, 128, 16, 128).transpose(0, 2, 1, 3).astype(BF16_NP))
        per_hg.append((wqk4, wv3, wp4))

    in_maps = []
    for core in range(N_CORES):
        b, hg = core // 2, core % 2
        wqk4, wv3, wp4 = per_hg[hg]
        in_maps.append({
            "xT": xT_b[b], "wqk4": wqk4, "wv3": wv3, "wp4": wp4,
            "cosT": cosT, "sinS": sinS, "ones": ones,
        })
    return in_maps


_NC_CACHE = {}


def get_nc(t=T):
    if t not in _NC_CACHE:
        _NC_CACHE[t] = build_nc(t)
    return _NC_CACHE[t]


def kernel(x, cos, sin, W_attn, W_proj):
    in_maps = make_in_maps(x, cos, sin, W_attn, W_proj)
    nc = get_nc(x.shape[1])
    res = run_bass_kernel_spmd(nc, in_maps, list(range(N_CORES))).results
    out = np.empty((B, x.shape[1], C), np.float32)
    for b in range(B):
        out[b] = (res[2 * b]["OT"] + res[2 * b + 1]["OT"]).T
    return out
